# revision 23
# baseline (speedup 1.0000x reference)
"""TRN2 Bass kernel for nn_DCABlock (1x1 convs + ECA channel attention + dual softmax).

Self-contained: hardcodes shapes for x:(16,2048,32,32) fp32.
Strategy: pure data parallelism — 2 samples per core on 8 NeuronCores.

Math (per sample, X = x[b] as (C,N) with N=h*w=1024, IC=C/2=1024):
  xphi = w_phi @ X                                 (IC,N)
  Q    = xphi * (1 + sigmoid(conv1d_k5(mean_n xphi)))   [ECA]
  S    = Q^T Q   (symmetric)                       (N,N)
  R    = rowsoftmax(S)           == sm^T (sm = softmax(S, axis=0))
  AT   = Q @ R                   == A^T            (IC,N)
  E2   = exp(AT - rowmax(AT)); rsU = rowsum(E2)    [sm2^T = E2/rsU]
  BT   = (E2^T @ Q) * (1/rsU per row)              (IC,N)
  out  = w_mask @ (AT + BT) + X                    (C,N)
(The reference's theta/eca_k branch is dead code and skipped.)

All large matmuls run as float32r (full PE rate, ~11-bit mantissa rounding,
fp32 PSUM accumulation); end-to-end error vs fp32 reference ~2e-4 scale-relative.
"""
import numpy as np

_C = 2048
_IC = 1024
_N = 1024
_H = 32
_NCORES = 8
_SPC = 2           # samples per core
_KECA = 5

_PROG = []


def _make_bands(wq):
    """(128, 3*128) fp32: band blocks so that the cross-channel ECA conv becomes
    24 tiny PE matmuls on the per-tile rowsum vector Y (128,8).

    s_logit[t*128+a] = sum_dt sum_p B[p, (dt+1)*128+a] * Y[p, t+dt]
    B[p, (dt+1)*128+a] = wq[p - a + 128*dt + 2] / N   (zero outside [0,5))
    """
    bands = np.zeros((128, 3 * 128), np.float32)
    p = np.arange(128)[:, None]
    a = np.arange(128)[None, :]
    for dt in (-1, 0, 1):
        j = p - a + 128 * dt + 2
        m = (j >= 0) & (j < _KECA)
        blk = np.zeros((128, 128), np.float32)
        blk[m] = (wq[np.clip(j, 0, _KECA - 1)] / _N)[m]
        bands[:, (dt + 1) * 128:(dt + 2) * 128] = blk
    return bands


def _build():
    if _PROG:
        return _PROG[0]
    import concourse.mybir as mybir
    import concourse.tile as tile
    from concourse import bacc
    from concourse.masks import make_identity

    f32 = mybir.dt.float32
    f32r = mybir.dt.float32r
    AX = mybir.AxisListType.X
    MAX = mybir.AluOpType.max
    EXP = mybir.ActivationFunctionType.Exp
    CPY = mybir.ActivationFunctionType.Copy

    nc = bacc.Bacc("TRN2", target_bir_lowering=False, debug=False,
                   num_devices=_NCORES)
    x_t = nc.dram_tensor("x", [_SPC, _C, _N], f32, kind="ExternalInput").ap()
    wphi_t = nc.dram_tensor("wphi", [8, 128, 16, 128], f32r,
                            kind="ExternalInput").ap()
    wmask_t = nc.dram_tensor("wmask", [16, 128, 8, 128], f32r,
                             kind="ExternalInput").ap()
    bands_t = nc.dram_tensor("bands", [128, 3 * 128], f32,
                             kind="ExternalInput").ap()
    out_t = nc.dram_tensor("out", [_SPC, _C, _N], f32, kind="ExternalOutput").ap()

    with tile.TileContext(nc) as tc:
        from contextlib import ExitStack
        ctx = ExitStack()
        with ctx:
            cst = ctx.enter_context(tc.tile_pool(name="cst", bufs=1))
            sml = ctx.enter_context(tc.tile_pool(name="sml", bufs=2))
            w1p = ctx.enter_context(tc.tile_pool(name="w1p", bufs=1))
            ap_ = ctx.enter_context(tc.tile_pool(name="apl", bufs=1))
            bp_ = ctx.enter_context(tc.tile_pool(name="bpl", bufs=1))
            dp_ = ctx.enter_context(tc.tile_pool(name="dpl", bufs=1))
            wcp = ctx.enter_context(tc.tile_pool(name="wcp", bufs=4))
            xrp = ctx.enter_context(tc.tile_pool(name="xrp", bufs=3))
            psa = ctx.enter_context(tc.tile_pool(name="psa", bufs=3, space="PSUM"))
            pst = ctx.enter_context(tc.tile_pool(name="pst", bufs=2, space="PSUM"))

            bands = cst.tile([128, 3 * 128], f32, tag="bands", name="bands_sb")
            nc.sync.dma_start(bands[:], bands_t[:])
            ident = cst.tile([128, 128], f32, tag="ident", name="ident_sb")
            make_identity(nc, ident[:])

            def transpose_8x8(src, dst, s, lbl, g_outer=False):
                """dst[:, t*1024 + d] = src[d-tile layout] transposed per 128x128 block.
                src/dst are (128, 8192) f32r tiles in the standard tiled layout.
                g_outer=True orders groups so early transposes only need the
                first half of src's column tiles (for src produced tile-by-tile)."""
                pairs = [(t, g) for g in range(2) for t in range(8)] if g_outer \
                    else [(t, g) for t in range(8) for g in range(2)]
                for t, g in pairs:
                    tp = pst.tile([128, 512], f32, tag="tp",
                                  name=f"tp_{lbl}{s}_{t}_{g}")
                    for j in range(4):
                        dtile = g * 4 + j
                        blk = src[:, dtile * 1024 + t * 128:
                                  dtile * 1024 + t * 128 + 128].bitcast(f32)
                        nc.tensor.transpose(tp[:, j * 128:(j + 1) * 128],
                                            blk, ident[:])
                    nc.vector.tensor_copy(dst[:, t * 1024 + g * 512:
                                               t * 1024 + (g + 1) * 512], tp[:])

            nxt = {}  # cross-sample prefetch state: wp tiles of sample s+1
            # One persistent work tile across samples: region reuse is tracked
            # by address (X -> [R | AT] -> adds), which lets next-sample X
            # prefetch start as soon as only the touched region is dead.
            w1 = w1p.tile([128, 16384], f32r, tag="w1", name="w1")

            def emit_x_load(s, ct):
                nc.sync.dma_start(
                    w1[:, ct * 1024:(ct + 1) * 1024],
                    x_t[s, ct * 128:(ct + 1) * 128, :].bitcast(f32r))

            def emit_wp(s, mt, split_first=False):
                wp = wcp.tile([128, 2048], f32r, tag="wcol", name=f"wp{s}_{mt}")
                src = wphi_t[mt].rearrange("p k m -> p (k m)")
                if split_first:
                    # land the first-consumed half first
                    nc.sync.dma_start(wp[:, 0:1024], src[:, 0:1024])
                    nc.sync.dma_start(wp[:, 1024:2048], src[:, 1024:2048])
                else:
                    nc.sync.dma_start(wp[:], src)
                return wp

            for s in range(_SPC):
                # ---- first phi weights, then X (upper half first) ----
                if s in nxt:
                    wps = nxt.pop(s)  # X fully prefetched during s-1
                else:
                    # cold start: interleave X tiles with phi-weight halves so
                    # the first matmuls aren't queued behind 2MB of weights
                    emit_x_load(s, 0)
                    wp0 = wcp.tile([128, 2048], f32r, tag="wcol", name=f"wp{s}_0")
                    w0src = wphi_t[0].rearrange("p k m -> p (k m)")
                    nc.sync.dma_start(wp0[:, 0:512], w0src[:, 0:512])
                    emit_x_load(s, 1)
                    nc.sync.dma_start(wp0[:, 512:1024], w0src[:, 512:1024])
                    emit_x_load(s, 2)
                    nc.sync.dma_start(wp0[:, 1024:2048], w0src[:, 1024:2048])
                    emit_x_load(s, 3)
                    wp1 = wcp.tile([128, 2048], f32r, tag="wcol", name=f"wp{s}_1")
                    w1src = wphi_t[1].rearrange("p k m -> p (k m)")
                    nc.sync.dma_start(wp1[:, 0:1024], w1src[:, 0:1024])
                    emit_x_load(s, 4)
                    nc.sync.dma_start(wp1[:, 1024:2048], w1src[:, 1024:2048])
                    for ct in range(5, 16):
                        emit_x_load(s, ct)
                    wps = {0: wp0, 1: wp1}

                # ---- phi: xphi[mt] = sum_kt wphi(kt,mt)^T @ X[kt] ----
                # ECA is pipelined per channel-tile: column t of the band-conv
                # needs only Y cols t-1..t+1, so it runs under later phi mms.
                xphi = ap_.tile([128, 8192], f32, tag="A", name=f"xphi{s}")
                Y = sml.tile([128, 8], f32, tag="Y", name=f"Y{s}")
                sp = pst.tile([128, 512], f32, tag="tp", name=f"eca{s}")
                sig = sml.tile([128, 8], f32, tag="sig", name=f"sig{s}")
                Qm = bp_.tile([128, 8192], f32r, tag="B", name=f"Qm{s}")

                def emit_eca_col(t):
                    steps = [dt for dt in (-1, 0, 1) if 0 <= t + dt < 8]
                    for i, dt in enumerate(steps):
                        nc.tensor.matmul(
                            sp[:, t:t + 1],
                            bands[:, (dt + 1) * 128:(dt + 2) * 128],
                            Y[:, t + dt:t + dt + 1],
                            start=(i == 0), stop=(i == len(steps) - 1))
                    sc = sig[:, t:t + 1]
                    nc.scalar.activation(sc, sp[:, t:t + 1], EXP, scale=-1.0)
                    nc.vector.tensor_scalar_add(sc, sc, 1.0)
                    nc.vector.reciprocal(sc, sc)
                    nc.vector.tensor_scalar_add(sc, sc, 1.0)
                    nc.scalar.activation(Qm[:, t * 1024:(t + 1) * 1024],
                                         xphi[:, t * 1024:(t + 1) * 1024],
                                         CPY, scale=sc)

                for mt in range(8):
                    wp = wps.pop(mt)
                    if mt + 2 < 8:
                        wps[mt + 2] = emit_wp(s, mt + 2)
                    acc = psa.tile([128, 1024], f32, tag="acc", name=f"phiacc{s}_{mt}")
                    for i in range(16):
                        for ch in range(2):
                            nc.tensor.matmul(
                                acc[:, ch * 512:(ch + 1) * 512],
                                wp[:, i * 128:(i + 1) * 128],
                                w1[:, i * 1024 + ch * 512: i * 1024 + (ch + 1) * 512],
                                start=(i == 0), stop=(i == 15))
                    nc.scalar.activation(xphi[:, mt * 1024:(mt + 1) * 1024], acc[:],
                                         CPY, accum_out=Y[:, mt:mt + 1])
                    if mt >= 1:
                        emit_eca_col(mt - 1)
                emit_eca_col(7)

                # ---- S[nt] = sum_t Qm[t][:,nt]^T @ Qm[t]; R = rowsoftmax(S) ----
                rs1 = sml.tile([128, 8], f32, tag="rs1", name=f"rs1{s}")
                for nt in range(8):
                    acc = psa.tile([128, 1024], f32, tag="acc", name=f"sacc{s}_{nt}")
                    for t in range(8):
                        lhsT = Qm[:, t * 1024 + nt * 128: t * 1024 + nt * 128 + 128]
                        for ch in range(2):
                            nc.tensor.matmul(
                                acc[:, ch * 512:(ch + 1) * 512], lhsT,
                                Qm[:, t * 1024 + ch * 512: t * 1024 + (ch + 1) * 512],
                                start=(t == 0), stop=(t == 7))
                    nm = sml.tile([128, 1], f32, tag="nm", name=f"nm{s}_{nt}")
                    nc.vector.tensor_reduce(nm[:], acc[:], axis=AX, op=MAX,
                                            negate=True)
                    rsl = w1[:, nt * 1024:(nt + 1) * 1024]
                    nc.scalar.activation(rsl, acc[:], EXP, bias=nm[:],
                                         accum_out=rs1[:, nt:nt + 1])
                    rc = sml.tile([128, 1], f32, tag="rc", name=f"rc{s}_{nt}")
                    nc.vector.reciprocal(rc[:], rs1[:, nt:nt + 1])
                    nc.vector.tensor_scalar_mul(rsl, rsl, rc[:])

                # ---- QT = Qm^T ----
                QT = dp_.tile([128, 8192], f32r, tag="D", name=f"QT{s}")
                transpose_8x8(Qm, QT, s, "qt")

                # ---- AT[mt] = sum_t QT[t][:,mt]^T @ R[t]; E2 = exp(AT - rowmax) ----
                rsU = sml.tile([128, 8], f32, tag="rsU", name=f"rsU{s}")
                E2 = ap_.tile([128, 8192], f32r, tag="A", name=f"E2_{s}")
                for mt in range(8):
                    acc = psa.tile([128, 1024], f32, tag="acc", name=f"atacc{s}_{mt}")
                    for t in range(8):
                        lhsT = QT[:, t * 1024 + mt * 128: t * 1024 + mt * 128 + 128]
                        for ch in range(2):
                            nc.tensor.matmul(
                                acc[:, ch * 512:(ch + 1) * 512], lhsT,
                                w1[:, t * 1024 + ch * 512: t * 1024 + (ch + 1) * 512],
                                start=(t == 0), stop=(t == 7))
                    ats = w1[:, 8192 + mt * 1024: 8192 + (mt + 1) * 1024]
                    nc.scalar.copy(ats, acc[:])
                    # |AT| <= ~max|Q| (attention-averaged), so exp needs no
                    # max subtraction; normalization divides it out exactly.
                    nc.scalar.activation(E2[:, mt * 1024:(mt + 1) * 1024], acc[:],
                                         EXP, accum_out=rsU[:, mt:mt + 1])
                recU = sml.tile([128, 8], f32, tag="recU", name=f"recU{s}")
                nc.vector.reciprocal(recU[:], rsU[:])

                # ---- E2T = E2^T ----
                E2T = dp_.tile([128, 8192], f32r, tag="D", name=f"E2T{s}")
                transpose_8x8(E2, E2T, s, "et", g_outer=True)

                # ---- BT[dt] = sum_t E2T[t][:,dt]^T @ Qm[t]; add = AT + BT/rsU ----
                # Next-sample lower-X deps (R region) cleared at AT end: emit
                # now so the loads flow during the DMA-idle BT window.
                if s + 1 < _SPC:
                    for ct in range(8):
                        emit_x_load(s + 1, ct)
                addt = ap_.tile([128, 8192], f32r, tag="A", name=f"add{s}")
                for dt in range(8):
                    acc = psa.tile([128, 1024], f32, tag="acc", name=f"btacc{s}_{dt}")
                    for t in range(8):
                        lhsT = E2T[:, t * 1024 + dt * 128: t * 1024 + dt * 128 + 128]
                        for ch in range(2):
                            nc.tensor.matmul(
                                acc[:, ch * 512:(ch + 1) * 512], lhsT,
                                Qm[:, t * 1024 + ch * 512: t * 1024 + (ch + 1) * 512],
                                start=(t == 0), stop=(t == 7))
                    adds = addt[:, dt * 1024:(dt + 1) * 1024]
                    nc.vector.tensor_scalar_mul(adds, acc[:], recU[:, dt:dt + 1])
                    nc.vector.tensor_add(
                        adds, adds,
                        w1[:, 8192 + dt * 1024: 8192 + (dt + 1) * 1024])

                # ---- mask[ct] = sum_kt wmask(kt,ct)^T @ add[kt]; out = mask + x ----
                # Pre-mask prefetch: first mask weights + x-residual tiles, and
                # next-sample upper-X (deps clear per-slab during BT drains).
                wms, xts = {}, {}
                for ct in range(4):
                    wms[ct] = wcp.tile([128, 1024], f32r, tag="wcol",
                                       name=f"wm{s}_{ct}")
                    nc.sync.dma_start(wms[ct][:],
                                      wmask_t[ct].rearrange("p k m -> p (k m)"))
                for ct in range(3):
                    xts[ct] = xrp.tile([128, 1024], f32, tag="xr", name=f"xr{s}_{ct}")
                    nc.sync.dma_start(xts[ct][:], x_t[s, ct * 128:(ct + 1) * 128, :])
                if s + 1 < _SPC:
                    for ct in range(8, 16):
                        emit_x_load(s + 1, ct)
                for ct in range(16):
                    wm = wms.pop(ct)
                    if ct + 4 < 16:
                        wms[ct + 4] = wcp.tile([128, 1024], f32r, tag="wcol",
                                               name=f"wm{s}_{ct + 4}")
                        nc.sync.dma_start(wms[ct + 4][:],
                                          wmask_t[ct + 4].rearrange("p k m -> p (k m)"))
                    xt = xts.pop(ct)
                    if ct + 3 < 16:
                        xts[ct + 3] = xrp.tile([128, 1024], f32, tag="xr",
                                               name=f"xr{s}_{ct + 3}")
                        nc.sync.dma_start(xts[ct + 3][:],
                                          x_t[s, (ct + 3) * 128:(ct + 4) * 128, :])
                    acc = psa.tile([128, 1024], f32, tag="acc", name=f"mkacc{s}_{ct}")
                    for kt in range(8):
                        for ch in range(2):
                            nc.tensor.matmul(
                                acc[:, ch * 512:(ch + 1) * 512],
                                wm[:, kt * 128:(kt + 1) * 128],
                                addt[:, kt * 1024 + ch * 512:
                                     kt * 1024 + (ch + 1) * 512],
                                start=(kt == 0), stop=(kt == 7))
                    nc.vector.tensor_add(xt[:], acc[:], xt[:])
                    nc.scalar.dma_start(out_t[s, ct * 128:(ct + 1) * 128, :], xt[:])
                if s + 1 < _SPC:
                    nxt[s + 1] = {mt: emit_wp(s + 1, mt) for mt in range(2)}

    nc.compile()
    _PROG.append(nc)
    return nc


def kernel(x, w_phi, w_eca_q, w_theta, w_eca_k, w_mask):
    from concourse.bass_utils import run_bass_kernel_spmd

    x = np.asarray(x, np.float32)
    w_phi = np.asarray(w_phi, np.float32)
    w_mask = np.asarray(w_mask, np.float32)
    w_eca_q = np.asarray(w_eca_q, np.float32)

    # host-side weight re-layouts (tiled for efficient per-partition DMA)
    # wphi[mt, p, kt, m] = w_phi[mt*128+m, kt*128+p]
    wphi_l = np.ascontiguousarray(
        w_phi.reshape(8, 128, 16, 128).transpose(0, 3, 2, 1))
    # wmask[ct, p, kt, m] = w_mask[ct*128+m, kt*128+p]
    wmask_l = np.ascontiguousarray(
        w_mask.reshape(16, 128, 8, 128).transpose(0, 3, 2, 1))
    bands = _make_bands(w_eca_q)

    xs = x.reshape(_NCORES, _SPC, _C, _N)
    nc = _build()
    in_maps = [{"x": np.ascontiguousarray(xs[i]), "wphi": wphi_l,
                "wmask": wmask_l, "bands": bands} for i in range(_NCORES)]
    res = run_bass_kernel_spmd(nc, in_maps, list(range(_NCORES)))
    out = np.stack([res.results[i]["out"] for i in range(_NCORES)])
    return out.reshape(_NCORES * _SPC, _C, _H, _H)


# revision 24
# speedup vs baseline: 170.6661x; 170.6661x over previous
"""TRN2 Bass kernel for nn_DCABlock (1x1 convs + ECA channel attention + dual softmax).

Self-contained: hardcodes shapes for x:(16,2048,32,32) fp32.
Strategy: pure data parallelism — 2 samples per core on 8 NeuronCores.

Math (per sample, X = x[b] as (C,N) with N=h*w=1024, IC=C/2=1024):
  xphi = w_phi @ X                                 (IC,N)
  Q    = xphi * (1 + sigmoid(conv1d_k5(mean_n xphi)))   [ECA]
  S    = Q^T Q   (symmetric)                       (N,N)
  R    = rowsoftmax(S)           == sm^T (sm = softmax(S, axis=0))
  AT   = Q @ R                   == A^T            (IC,N)
  E2   = exp(AT - rowmax(AT)); rsU = rowsum(E2)    [sm2^T = E2/rsU]
  BT   = (E2^T @ Q) * (1/rsU per row)              (IC,N)
  out  = w_mask @ (AT + BT) + X                    (C,N)
(The reference's theta/eca_k branch is dead code and skipped.)

All large matmuls run as float32r (full PE rate, ~11-bit mantissa rounding,
fp32 PSUM accumulation); end-to-end error vs fp32 reference ~2e-4 scale-relative.
"""
import numpy as np

_C = 2048
_IC = 1024
_N = 1024
_H = 32
_NCORES = 8
_SPC = 2           # samples per core
_KECA = 5

_PROG = {}


def _make_bands(wq):
    """(128, 3*128) fp32: band blocks so that the cross-channel ECA conv becomes
    24 tiny PE matmuls on the per-tile rowsum vector Y (128,8).

    s_logit[t*128+a] = sum_dt sum_p B[p, (dt+1)*128+a] * Y[p, t+dt]
    B[p, (dt+1)*128+a] = wq[p - a + 128*dt + 2] / N   (zero outside [0,5))
    """
    bands = np.zeros((128, 3 * 128), np.float32)
    p = np.arange(128)[:, None]
    a = np.arange(128)[None, :]
    for dt in (-1, 0, 1):
        j = p - a + 128 * dt + 2
        m = (j >= 0) & (j < _KECA)
        blk = np.zeros((128, 128), np.float32)
        blk[m] = (wq[np.clip(j, 0, _KECA - 1)] / _N)[m]
        bands[:, (dt + 1) * 128:(dt + 2) * 128] = blk
    return bands


def _build(reps=1):
    if reps in _PROG:
        return _PROG[reps]
    import concourse.mybir as mybir
    import concourse.tile as tile
    from concourse import bacc
    from concourse.masks import make_identity

    f32 = mybir.dt.float32
    f32r = mybir.dt.float32r
    AX = mybir.AxisListType.X
    MAX = mybir.AluOpType.max
    EXP = mybir.ActivationFunctionType.Exp
    CPY = mybir.ActivationFunctionType.Copy

    nc = bacc.Bacc("TRN2", target_bir_lowering=False, debug=False,
                   num_devices=_NCORES)
    x_t = nc.dram_tensor("x", [_SPC, _C, _N], f32, kind="ExternalInput").ap()
    wphi_t = nc.dram_tensor("wphi", [8, 128, 16, 128], f32r,
                            kind="ExternalInput").ap()
    wmask_t = nc.dram_tensor("wmask", [16, 128, 8, 128], f32r,
                             kind="ExternalInput").ap()
    bands_t = nc.dram_tensor("bands", [128, 3 * 128], f32,
                             kind="ExternalInput").ap()
    out_t = nc.dram_tensor("out", [_SPC, _C, _N], f32, kind="ExternalOutput").ap()

    with tile.TileContext(nc) as tc:
        from contextlib import ExitStack
        ctx = ExitStack()
        with ctx:
            cst = ctx.enter_context(tc.tile_pool(name="cst", bufs=1))
            sml = ctx.enter_context(tc.tile_pool(name="sml", bufs=2))
            w1p = ctx.enter_context(tc.tile_pool(name="w1p", bufs=1))
            ap_ = ctx.enter_context(tc.tile_pool(name="apl", bufs=1))
            bp_ = ctx.enter_context(tc.tile_pool(name="bpl", bufs=1))
            dp_ = ctx.enter_context(tc.tile_pool(name="dpl", bufs=1))
            wcp = ctx.enter_context(tc.tile_pool(name="wcp", bufs=4))
            xrp = ctx.enter_context(tc.tile_pool(name="xrp", bufs=3))
            psa = ctx.enter_context(tc.tile_pool(name="psa", bufs=3, space="PSUM"))
            pst = ctx.enter_context(tc.tile_pool(name="pst", bufs=2, space="PSUM"))

            bands = cst.tile([128, 3 * 128], f32, tag="bands", name="bands_sb")
            nc.sync.dma_start(bands[:], bands_t[:])
            ident = cst.tile([128, 128], f32, tag="ident", name="ident_sb")
            make_identity(nc, ident[:])

            def transpose_8x8(src, dst, u, lbl, g_outer=False):
                """dst[:, t*1024 + d] = src[d-tile layout] transposed per 128x128 block.
                src/dst are (128, 8192) f32r tiles in the standard tiled layout.
                g_outer=True orders groups so early transposes only need the
                first half of src's column tiles (for src produced tile-by-tile)."""
                pairs = [(t, g) for g in range(2) for t in range(8)] if g_outer \
                    else [(t, g) for t in range(8) for g in range(2)]
                for t, g in pairs:
                    tp = pst.tile([128, 512], f32, tag="tp",
                                  name=f"tp_{lbl}{u}_{t}_{g}")
                    for j in range(4):
                        dtile = g * 4 + j
                        blk = src[:, dtile * 1024 + t * 128:
                                  dtile * 1024 + t * 128 + 128].bitcast(f32)
                        nc.tensor.transpose(tp[:, j * 128:(j + 1) * 128],
                                            blk, ident[:])
                    nc.vector.tensor_copy(dst[:, t * 1024 + g * 512:
                                               t * 1024 + (g + 1) * 512], tp[:])

            nxt = {}  # cross-sample prefetch state: wp tiles of sample s+1
            # One persistent work tile across samples: region reuse is tracked
            # by address (X -> [R | AT] -> adds), which lets next-sample X
            # prefetch start as soon as only the touched region is dead.
            w1 = w1p.tile([128, 16384], f32r, tag="w1", name="w1")

            def emit_x_load(s, ct):
                nc.sync.dma_start(
                    w1[:, ct * 1024:(ct + 1) * 1024],
                    x_t[s, ct * 128:(ct + 1) * 128, :].bitcast(f32r))

            def emit_wp(u, mt, split_first=False):
                wp = wcp.tile([128, 2048], f32r, tag="wcol", name=f"wp{u}_{mt}")
                src = wphi_t[mt].rearrange("p k m -> p (k m)")
                if split_first:
                    # land the first-consumed half first
                    nc.sync.dma_start(wp[:, 0:1024], src[:, 0:1024])
                    nc.sync.dma_start(wp[:, 1024:2048], src[:, 1024:2048])
                else:
                    nc.sync.dma_start(wp[:], src)
                return wp

            seq = [sp for _ in range(reps) for sp in range(_SPC)]
            for u, s in enumerate(seq):
                s_nxt = seq[u + 1] if u + 1 < len(seq) else None
                # ---- first phi weights, then X (upper half first) ----
                if u in nxt:
                    wps = nxt.pop(u)  # X fully prefetched during prior block
                else:
                    # cold start: interleave X tiles with phi-weight halves so
                    # the first matmuls aren't queued behind 2MB of weights
                    emit_x_load(s, 0)
                    wp0 = wcp.tile([128, 2048], f32r, tag="wcol", name=f"wp{u}_0")
                    w0src = wphi_t[0].rearrange("p k m -> p (k m)")
                    nc.sync.dma_start(wp0[:, 0:512], w0src[:, 0:512])
                    emit_x_load(s, 1)
                    nc.sync.dma_start(wp0[:, 512:1024], w0src[:, 512:1024])
                    emit_x_load(s, 2)
                    nc.sync.dma_start(wp0[:, 1024:2048], w0src[:, 1024:2048])
                    emit_x_load(s, 3)
                    wp1 = wcp.tile([128, 2048], f32r, tag="wcol", name=f"wp{u}_1")
                    w1src = wphi_t[1].rearrange("p k m -> p (k m)")
                    nc.sync.dma_start(wp1[:, 0:1024], w1src[:, 0:1024])
                    emit_x_load(s, 4)
                    nc.sync.dma_start(wp1[:, 1024:2048], w1src[:, 1024:2048])
                    for ct in range(5, 16):
                        emit_x_load(s, ct)
                    wps = {0: wp0, 1: wp1}

                # ---- phi: xphi[mt] = sum_kt wphi(kt,mt)^T @ X[kt] ----
                # ECA is pipelined per channel-tile: column t of the band-conv
                # needs only Y cols t-1..t+1, so it runs under later phi mms.
                xphi = ap_.tile([128, 8192], f32, tag="A", name=f"xphi{u}")
                Y = sml.tile([128, 8], f32, tag="Y", name=f"Y{u}")
                sp = pst.tile([128, 512], f32, tag="tp", name=f"eca{u}")
                sig = sml.tile([128, 8], f32, tag="sig", name=f"sig{u}")
                Qm = bp_.tile([128, 8192], f32r, tag="B", name=f"Qm{u}")

                def emit_eca_col(t):
                    steps = [dt for dt in (-1, 0, 1) if 0 <= t + dt < 8]
                    for i, dt in enumerate(steps):
                        nc.tensor.matmul(
                            sp[:, t:t + 1],
                            bands[:, (dt + 1) * 128:(dt + 2) * 128],
                            Y[:, t + dt:t + dt + 1],
                            start=(i == 0), stop=(i == len(steps) - 1))
                    sc = sig[:, t:t + 1]
                    nc.scalar.activation(sc, sp[:, t:t + 1], EXP, scale=-1.0)
                    nc.vector.tensor_scalar_add(sc, sc, 1.0)
                    nc.vector.reciprocal(sc, sc)
                    nc.vector.tensor_scalar_add(sc, sc, 1.0)
                    nc.scalar.activation(Qm[:, t * 1024:(t + 1) * 1024],
                                         xphi[:, t * 1024:(t + 1) * 1024],
                                         CPY, scale=sc)

                for mt in range(8):
                    wp = wps.pop(mt)
                    if mt + 2 < 8:
                        wps[mt + 2] = emit_wp(u, mt + 2)
                    acc = psa.tile([128, 1024], f32, tag="acc", name=f"phiacc{u}_{mt}")
                    for i in range(16):
                        for ch in range(2):
                            nc.tensor.matmul(
                                acc[:, ch * 512:(ch + 1) * 512],
                                wp[:, i * 128:(i + 1) * 128],
                                w1[:, i * 1024 + ch * 512: i * 1024 + (ch + 1) * 512],
                                start=(i == 0), stop=(i == 15))
                    nc.scalar.activation(xphi[:, mt * 1024:(mt + 1) * 1024], acc[:],
                                         CPY, accum_out=Y[:, mt:mt + 1])
                    if mt >= 1:
                        emit_eca_col(mt - 1)
                emit_eca_col(7)

                # ---- S[nt] = sum_t Qm[t][:,nt]^T @ Qm[t]; R = rowsoftmax(S) ----
                rs1 = sml.tile([128, 8], f32, tag="rs1", name=f"rs1{u}")
                for nt in range(8):
                    acc = psa.tile([128, 1024], f32, tag="acc", name=f"sacc{u}_{nt}")
                    for t in range(8):
                        lhsT = Qm[:, t * 1024 + nt * 128: t * 1024 + nt * 128 + 128]
                        for ch in range(2):
                            nc.tensor.matmul(
                                acc[:, ch * 512:(ch + 1) * 512], lhsT,
                                Qm[:, t * 1024 + ch * 512: t * 1024 + (ch + 1) * 512],
                                start=(t == 0), stop=(t == 7))
                    nm = sml.tile([128, 1], f32, tag="nm", name=f"nm{u}_{nt}")
                    nc.vector.tensor_reduce(nm[:], acc[:], axis=AX, op=MAX,
                                            negate=True)
                    rsl = w1[:, nt * 1024:(nt + 1) * 1024]
                    nc.scalar.activation(rsl, acc[:], EXP, bias=nm[:],
                                         accum_out=rs1[:, nt:nt + 1])
                    rc = sml.tile([128, 1], f32, tag="rc", name=f"rc{u}_{nt}")
                    nc.vector.reciprocal(rc[:], rs1[:, nt:nt + 1])
                    nc.vector.tensor_scalar_mul(rsl, rsl, rc[:])

                # ---- QT = Qm^T ----
                QT = dp_.tile([128, 8192], f32r, tag="D", name=f"QT{u}")
                transpose_8x8(Qm, QT, u, "qt")

                # ---- AT[mt] = sum_t QT[t][:,mt]^T @ R[t]; E2 = exp(AT - rowmax) ----
                rsU = sml.tile([128, 8], f32, tag="rsU", name=f"rsU{u}")
                E2 = ap_.tile([128, 8192], f32r, tag="A", name=f"E2_{u}")
                for mt in range(8):
                    acc = psa.tile([128, 1024], f32, tag="acc", name=f"atacc{u}_{mt}")
                    for t in range(8):
                        lhsT = QT[:, t * 1024 + mt * 128: t * 1024 + mt * 128 + 128]
                        for ch in range(2):
                            nc.tensor.matmul(
                                acc[:, ch * 512:(ch + 1) * 512], lhsT,
                                w1[:, t * 1024 + ch * 512: t * 1024 + (ch + 1) * 512],
                                start=(t == 0), stop=(t == 7))
                    ats = w1[:, 8192 + mt * 1024: 8192 + (mt + 1) * 1024]
                    nc.scalar.copy(ats, acc[:])
                    # |AT| <= ~max|Q| (attention-averaged), so exp needs no
                    # max subtraction; normalization divides it out exactly.
                    nc.scalar.activation(E2[:, mt * 1024:(mt + 1) * 1024], acc[:],
                                         EXP, accum_out=rsU[:, mt:mt + 1])
                recU = sml.tile([128, 8], f32, tag="recU", name=f"recU{u}")
                nc.vector.reciprocal(recU[:], rsU[:])

                # ---- E2T = E2^T ----
                E2T = dp_.tile([128, 8192], f32r, tag="D", name=f"E2T{u}")
                transpose_8x8(E2, E2T, u, "et", g_outer=True)

                # ---- BT[dt] = sum_t E2T[t][:,dt]^T @ Qm[t]; add = AT + BT/rsU ----
                # Next-sample lower-X deps (R region) cleared at AT end: emit
                # now so the loads flow during the DMA-idle BT window.
                if s_nxt is not None:
                    for ct in range(8):
                        emit_x_load(s_nxt, ct)
                addt = ap_.tile([128, 8192], f32r, tag="A", name=f"add{u}")
                for dt in range(8):
                    acc = psa.tile([128, 1024], f32, tag="acc", name=f"btacc{u}_{dt}")
                    for t in range(8):
                        lhsT = E2T[:, t * 1024 + dt * 128: t * 1024 + dt * 128 + 128]
                        for ch in range(2):
                            nc.tensor.matmul(
                                acc[:, ch * 512:(ch + 1) * 512], lhsT,
                                Qm[:, t * 1024 + ch * 512: t * 1024 + (ch + 1) * 512],
                                start=(t == 0), stop=(t == 7))
                    adds = addt[:, dt * 1024:(dt + 1) * 1024]
                    nc.vector.tensor_scalar_mul(adds, acc[:], recU[:, dt:dt + 1])
                    nc.vector.tensor_add(
                        adds, adds,
                        w1[:, 8192 + dt * 1024: 8192 + (dt + 1) * 1024])

                # ---- mask[ct] = sum_kt wmask(kt,ct)^T @ add[kt]; out = mask + x ----
                # Pre-mask prefetch: first mask weights + x-residual tiles, and
                # next-sample upper-X (deps clear per-slab during BT drains).
                wms, xts = {}, {}
                for ct in range(4):
                    wms[ct] = wcp.tile([128, 1024], f32r, tag="wcol",
                                       name=f"wm{u}_{ct}")
                    nc.sync.dma_start(wms[ct][:],
                                      wmask_t[ct].rearrange("p k m -> p (k m)"))
                for ct in range(3):
                    xts[ct] = xrp.tile([128, 1024], f32, tag="xr", name=f"xr{u}_{ct}")
                    nc.sync.dma_start(xts[ct][:], x_t[s, ct * 128:(ct + 1) * 128, :])
                if s_nxt is not None:
                    for ct in range(8, 16):
                        emit_x_load(s_nxt, ct)
                for ct in range(16):
                    wm = wms.pop(ct)
                    if ct + 4 < 16:
                        wms[ct + 4] = wcp.tile([128, 1024], f32r, tag="wcol",
                                               name=f"wm{u}_{ct + 4}")
                        nc.sync.dma_start(wms[ct + 4][:],
                                          wmask_t[ct + 4].rearrange("p k m -> p (k m)"))
                    xt = xts.pop(ct)
                    if ct + 3 < 16:
                        xts[ct + 3] = xrp.tile([128, 1024], f32, tag="xr",
                                               name=f"xr{u}_{ct + 3}")
                        nc.sync.dma_start(xts[ct + 3][:],
                                          x_t[s, (ct + 3) * 128:(ct + 4) * 128, :])
                    acc = psa.tile([128, 1024], f32, tag="acc", name=f"mkacc{u}_{ct}")
                    for kt in range(8):
                        for ch in range(2):
                            nc.tensor.matmul(
                                acc[:, ch * 512:(ch + 1) * 512],
                                wm[:, kt * 128:(kt + 1) * 128],
                                addt[:, kt * 1024 + ch * 512:
                                     kt * 1024 + (ch + 1) * 512],
                                start=(kt == 0), stop=(kt == 7))
                    nc.vector.tensor_add(xt[:], acc[:], xt[:])
                    nc.scalar.dma_start(out_t[s, ct * 128:(ct + 1) * 128, :], xt[:])
                if s_nxt is not None:
                    nxt[u + 1] = {mt: emit_wp(u + 1, mt) for mt in range(2)}

    nc.compile()
    _PROG[reps] = nc
    return nc


def kernel(x, w_phi, w_eca_q, w_theta, w_eca_k, w_mask):
    from concourse.bass_utils import run_bass_kernel_spmd

    x = np.asarray(x, np.float32)
    w_phi = np.asarray(w_phi, np.float32)
    w_mask = np.asarray(w_mask, np.float32)
    w_eca_q = np.asarray(w_eca_q, np.float32)

    # host-side weight re-layouts (tiled for efficient per-partition DMA)
    # wphi[mt, p, kt, m] = w_phi[mt*128+m, kt*128+p]
    wphi_l = np.ascontiguousarray(
        w_phi.reshape(8, 128, 16, 128).transpose(0, 3, 2, 1))
    # wmask[ct, p, kt, m] = w_mask[ct*128+m, kt*128+p]
    wmask_l = np.ascontiguousarray(
        w_mask.reshape(16, 128, 8, 128).transpose(0, 3, 2, 1))
    bands = _make_bands(w_eca_q)

    xs = x.reshape(_NCORES, _SPC, _C, _N)
    nc = _build()
    in_maps = [{"x": np.ascontiguousarray(xs[i]), "wphi": wphi_l,
                "wmask": wmask_l, "bands": bands} for i in range(_NCORES)]
    res = run_bass_kernel_spmd(nc, in_maps, list(range(_NCORES)))
    out = np.stack([res.results[i]["out"] for i in range(_NCORES)])
    return out.reshape(_NCORES * _SPC, _C, _H, _H)


# revision 29
# speedup vs baseline: 172.9555x; 1.0134x over previous
"""TRN2 Bass kernel for nn_DCABlock (1x1 convs + ECA channel attention + dual softmax).

Self-contained: hardcodes shapes for x:(16,2048,32,32) fp32.
Strategy: pure data parallelism — 2 samples per core on 8 NeuronCores.

Math (per sample, X = x[b] as (C,N) with N=h*w=1024, IC=C/2=1024):
  xphi = w_phi @ X                                 (IC,N)
  Q    = xphi * (1 + sigmoid(conv1d_k5(mean_n xphi)))   [ECA]
  S    = Q^T Q   (symmetric)                       (N,N)
  R    = rowsoftmax(S)           == sm^T (sm = softmax(S, axis=0))
  AT   = Q @ R                   == A^T            (IC,N)
  E2   = exp(AT - rowmax(AT)); rsU = rowsum(E2)    [sm2^T = E2/rsU]
  BT   = (E2^T @ Q) * (1/rsU per row)              (IC,N)
  out  = w_mask @ (AT + BT) + X                    (C,N)
(The reference's theta/eca_k branch is dead code and skipped.)

All large matmuls run as float32r (full PE rate, ~11-bit mantissa rounding,
fp32 PSUM accumulation); end-to-end error vs fp32 reference ~2e-4 scale-relative.
"""
import numpy as np

_C = 2048
_IC = 1024
_N = 1024
_H = 32
_NCORES = 8
_SPC = 2           # samples per core
_KECA = 5

_PROG = {}


def _make_bands(wq):
    """(128, 3*128) fp32: band blocks so that the cross-channel ECA conv becomes
    24 tiny PE matmuls on the per-tile rowsum vector Y (128,8).

    s_logit[t*128+a] = sum_dt sum_p B[p, (dt+1)*128+a] * Y[p, t+dt]
    B[p, (dt+1)*128+a] = wq[p - a + 128*dt + 2] / N   (zero outside [0,5))
    """
    bands = np.zeros((128, 3 * 128), np.float32)
    p = np.arange(128)[:, None]
    a = np.arange(128)[None, :]
    for dt in (-1, 0, 1):
        j = p - a + 128 * dt + 2
        m = (j >= 0) & (j < _KECA)
        blk = np.zeros((128, 128), np.float32)
        blk[m] = (wq[np.clip(j, 0, _KECA - 1)] / _N)[m]
        bands[:, (dt + 1) * 128:(dt + 2) * 128] = blk
    return bands


def _build(reps=1):
    if reps in _PROG:
        return _PROG[reps]
    import concourse.mybir as mybir
    import concourse.tile as tile
    from concourse import bacc
    from concourse.masks import make_identity

    f32 = mybir.dt.float32
    f32r = mybir.dt.float32r
    AX = mybir.AxisListType.X
    MAX = mybir.AluOpType.max
    EXP = mybir.ActivationFunctionType.Exp
    CPY = mybir.ActivationFunctionType.Copy

    nc = bacc.Bacc("TRN2", target_bir_lowering=False, debug=False,
                   num_devices=_NCORES)
    x_t = nc.dram_tensor("x", [_SPC, _C, _N], f32, kind="ExternalInput").ap()
    wphi_t = nc.dram_tensor("wphi", [8, 128, 16, 128], f32r,
                            kind="ExternalInput").ap()
    wmask_t = nc.dram_tensor("wmask", [16, 128, 8, 128], f32r,
                             kind="ExternalInput").ap()
    bands_t = nc.dram_tensor("bands", [128, 3 * 128], f32,
                             kind="ExternalInput").ap()
    out_t = nc.dram_tensor("out", [_SPC, _C, _N], f32, kind="ExternalOutput").ap()

    with tile.TileContext(nc) as tc:
        from contextlib import ExitStack
        ctx = ExitStack()
        with ctx:
            cst = ctx.enter_context(tc.tile_pool(name="cst", bufs=1))
            sml = ctx.enter_context(tc.tile_pool(name="sml", bufs=2))
            w1p = ctx.enter_context(tc.tile_pool(name="w1p", bufs=1))
            ap_ = ctx.enter_context(tc.tile_pool(name="apl", bufs=1))
            bp_ = ctx.enter_context(tc.tile_pool(name="bpl", bufs=1))
            dp_ = ctx.enter_context(tc.tile_pool(name="dpl", bufs=1))
            wcp = ctx.enter_context(tc.tile_pool(name="wcp", bufs=4))
            xrp = ctx.enter_context(tc.tile_pool(name="xrp", bufs=3))
            psa = ctx.enter_context(tc.tile_pool(name="psa", bufs=3, space="PSUM"))
            pst = ctx.enter_context(tc.tile_pool(name="pst", bufs=2, space="PSUM"))

            bands = cst.tile([128, 3 * 128], f32, tag="bands", name="bands_sb")
            nc.sync.dma_start(bands[:], bands_t[:])
            ident = cst.tile([128, 128], f32, tag="ident", name="ident_sb")
            make_identity(nc, ident[:])
            identr = cst.tile([128, 128], f32r, tag="identr", name="identr_sb")
            nc.vector.tensor_copy(identr[:], ident[:])

            def transpose_8x8(src, dst, u, lbl, g_outer=False):
                """dst[:, t*1024 + d] = src[d-tile layout] transposed per 128x128 block.
                src/dst are (128, 8192) f32r tiles in the standard tiled layout.
                g_outer=True orders groups so early transposes only need the
                first half of src's column tiles (for src produced tile-by-tile)."""
                pairs = [(t, g) for g in range(2) for t in range(8)] if g_outer \
                    else [(t, g) for t in range(8) for g in range(2)]
                for t, g in pairs:
                    tp = pst.tile([128, 512], f32r, tag="tp",
                                  name=f"tp_{lbl}{u}_{t}_{g}")
                    for j in range(4):
                        dtile = g * 4 + j
                        blk = src[:, dtile * 1024 + t * 128:
                                  dtile * 1024 + t * 128 + 128]
                        nc.tensor.transpose(tp[:, j * 128:(j + 1) * 128],
                                            blk, identr[:])
                    eng = nc.vector.tensor_copy if g == 0 else nc.scalar.copy
                    eng(dst[:, t * 1024 + g * 512:
                            t * 1024 + (g + 1) * 512], tp[:])

            nxt = {}  # cross-sample prefetch state: wp tiles of sample s+1
            # One persistent work tile across samples: region reuse is tracked
            # by address (X -> [R | AT] -> adds), which lets next-sample X
            # prefetch start as soon as only the touched region is dead.
            w1 = w1p.tile([128, 16384], f32r, tag="w1", name="w1")

            def emit_x_load(s, ct):
                nc.sync.dma_start(
                    w1[:, ct * 1024:(ct + 1) * 1024],
                    x_t[s, ct * 128:(ct + 1) * 128, :].bitcast(f32r))

            def emit_wp(u, mt, split_first=False):
                wp = wcp.tile([128, 2048], f32r, tag="wcol", name=f"wp{u}_{mt}")
                src = wphi_t[mt].rearrange("p k m -> p (k m)")
                if split_first:
                    # land the first-consumed half first
                    nc.sync.dma_start(wp[:, 0:1024], src[:, 0:1024])
                    nc.sync.dma_start(wp[:, 1024:2048], src[:, 1024:2048])
                else:
                    nc.sync.dma_start(wp[:], src)
                return wp

            seq = [sp for _ in range(reps) for sp in range(_SPC)]
            for u, s in enumerate(seq):
                s_nxt = seq[u + 1] if u + 1 < len(seq) else None
                # ---- first phi weights, then X (upper half first) ----
                if u in nxt:
                    wps = nxt.pop(u)  # X fully prefetched during prior block
                else:
                    # cold start: interleave X tiles with phi-weight halves so
                    # the first matmuls aren't queued behind 2MB of weights
                    emit_x_load(s, 0)
                    wp0 = wcp.tile([128, 2048], f32r, tag="wcol", name=f"wp{u}_0")
                    w0src = wphi_t[0].rearrange("p k m -> p (k m)")
                    nc.sync.dma_start(wp0[:, 0:512], w0src[:, 0:512])
                    emit_x_load(s, 1)
                    nc.sync.dma_start(wp0[:, 512:1024], w0src[:, 512:1024])
                    emit_x_load(s, 2)
                    nc.sync.dma_start(wp0[:, 1024:2048], w0src[:, 1024:2048])
                    emit_x_load(s, 3)
                    wp1 = wcp.tile([128, 2048], f32r, tag="wcol", name=f"wp{u}_1")
                    w1src = wphi_t[1].rearrange("p k m -> p (k m)")
                    nc.sync.dma_start(wp1[:, 0:1024], w1src[:, 0:1024])
                    emit_x_load(s, 4)
                    nc.sync.dma_start(wp1[:, 1024:2048], w1src[:, 1024:2048])
                    for ct in range(5, 16):
                        emit_x_load(s, ct)
                    wps = {0: wp0, 1: wp1}

                # ---- phi: xphi[mt] = sum_kt wphi(kt,mt)^T @ X[kt] ----
                # ECA is pipelined per channel-tile: column t of the band-conv
                # needs only Y cols t-1..t+1, so it runs under later phi mms.
                xphi = ap_.tile([128, 8192], f32, tag="A", name=f"xphi{u}")
                Y = sml.tile([128, 8], f32, tag="Y", name=f"Y{u}")
                sp = pst.tile([128, 512], f32, tag="tp", name=f"eca{u}")
                sig = sml.tile([128, 8], f32, tag="sig", name=f"sig{u}")
                Qm = bp_.tile([128, 8192], f32r, tag="B", name=f"Qm{u}")

                def emit_eca_col(t):
                    steps = [dt for dt in (-1, 0, 1) if 0 <= t + dt < 8]
                    for i, dt in enumerate(steps):
                        nc.tensor.matmul(
                            sp[:, t:t + 1],
                            bands[:, (dt + 1) * 128:(dt + 2) * 128],
                            Y[:, t + dt:t + dt + 1],
                            start=(i == 0), stop=(i == len(steps) - 1))
                    sc = sig[:, t:t + 1]
                    nc.scalar.activation(sc, sp[:, t:t + 1], EXP, scale=-1.0)
                    nc.vector.tensor_scalar_add(sc, sc, 1.0)
                    nc.vector.reciprocal(sc, sc)
                    nc.vector.tensor_scalar_add(sc, sc, 1.0)
                    nc.scalar.activation(Qm[:, t * 1024:(t + 1) * 1024],
                                         xphi[:, t * 1024:(t + 1) * 1024],
                                         CPY, scale=sc)

                for mt in range(8):
                    wp = wps.pop(mt)
                    if mt + 2 < 8:
                        wps[mt + 2] = emit_wp(u, mt + 2)
                    acc = psa.tile([128, 1024], f32, tag="acc", name=f"phiacc{u}_{mt}")
                    for i in range(16):
                        for ch in range(2):
                            nc.tensor.matmul(
                                acc[:, ch * 512:(ch + 1) * 512],
                                wp[:, i * 128:(i + 1) * 128],
                                w1[:, i * 1024 + ch * 512: i * 1024 + (ch + 1) * 512],
                                start=(i == 0), stop=(i == 15))
                    nc.scalar.activation(xphi[:, mt * 1024:(mt + 1) * 1024], acc[:],
                                         CPY, accum_out=Y[:, mt:mt + 1])
                    if mt >= 1:
                        emit_eca_col(mt - 1)
                emit_eca_col(7)

                # ---- S[nt] = sum_t Qm[t][:,nt]^T @ Qm[t]; R = rowsoftmax(S) ----
                rs1 = sml.tile([128, 8], f32, tag="rs1", name=f"rs1{u}")
                for nt in range(8):
                    acc = psa.tile([128, 1024], f32, tag="acc", name=f"sacc{u}_{nt}")
                    for t in range(8):
                        lhsT = Qm[:, t * 1024 + nt * 128: t * 1024 + nt * 128 + 128]
                        for ch in range(2):
                            nc.tensor.matmul(
                                acc[:, ch * 512:(ch + 1) * 512], lhsT,
                                Qm[:, t * 1024 + ch * 512: t * 1024 + (ch + 1) * 512],
                                start=(t == 0), stop=(t == 7))
                    nm = sml.tile([128, 1], f32, tag="nm", name=f"nm{u}_{nt}")
                    nc.vector.tensor_reduce(nm[:], acc[:], axis=AX, op=MAX,
                                            negate=True)
                    rsl = w1[:, nt * 1024:(nt + 1) * 1024]
                    nc.scalar.activation(rsl, acc[:], EXP, bias=nm[:],
                                         accum_out=rs1[:, nt:nt + 1])
                    rc = sml.tile([128, 1], f32, tag="rc", name=f"rc{u}_{nt}")
                    nc.vector.reciprocal(rc[:], rs1[:, nt:nt + 1])
                    nc.vector.tensor_scalar_mul(rsl, rsl, rc[:])

                # ---- QT = Qm^T ----
                QT = dp_.tile([128, 8192], f32r, tag="D", name=f"QT{u}")
                transpose_8x8(Qm, QT, u, "qt")

                # ---- AT[mt] = sum_t QT[t][:,mt]^T @ R[t]; E2 = exp(AT - rowmax) ----
                rsU = sml.tile([128, 8], f32, tag="rsU", name=f"rsU{u}")
                E2 = ap_.tile([128, 8192], f32r, tag="A", name=f"E2_{u}")
                for mt in range(8):
                    acc = psa.tile([128, 1024], f32, tag="acc", name=f"atacc{u}_{mt}")
                    for t in range(8):
                        lhsT = QT[:, t * 1024 + mt * 128: t * 1024 + mt * 128 + 128]
                        for ch in range(2):
                            nc.tensor.matmul(
                                acc[:, ch * 512:(ch + 1) * 512], lhsT,
                                w1[:, t * 1024 + ch * 512: t * 1024 + (ch + 1) * 512],
                                start=(t == 0), stop=(t == 7))
                    ats = w1[:, 8192 + mt * 1024: 8192 + (mt + 1) * 1024]
                    nc.scalar.copy(ats, acc[:])
                    # |AT| <= ~max|Q| (attention-averaged), so exp needs no
                    # max subtraction; normalization divides it out exactly.
                    nc.scalar.activation(E2[:, mt * 1024:(mt + 1) * 1024], acc[:],
                                         EXP, accum_out=rsU[:, mt:mt + 1])
                recU = sml.tile([128, 8], f32, tag="recU", name=f"recU{u}")
                nc.vector.reciprocal(recU[:], rsU[:])

                # ---- E2T = E2^T ----
                E2T = dp_.tile([128, 8192], f32r, tag="D", name=f"E2T{u}")
                transpose_8x8(E2, E2T, u, "et", g_outer=True)

                # ---- BT[dt] = sum_t E2T[t][:,dt]^T @ Qm[t]; add = AT + BT/rsU ----
                # Next-sample lower-X deps (R region) cleared at AT end: emit
                # now so the loads flow during the DMA-idle BT window.
                if s_nxt is not None:
                    for ct in range(8):
                        emit_x_load(s_nxt, ct)
                addt = ap_.tile([128, 8192], f32r, tag="A", name=f"add{u}")
                for dt in range(8):
                    acc = psa.tile([128, 1024], f32, tag="acc", name=f"btacc{u}_{dt}")
                    for t in range(8):
                        lhsT = E2T[:, t * 1024 + dt * 128: t * 1024 + dt * 128 + 128]
                        for ch in range(2):
                            nc.tensor.matmul(
                                acc[:, ch * 512:(ch + 1) * 512], lhsT,
                                Qm[:, t * 1024 + ch * 512: t * 1024 + (ch + 1) * 512],
                                start=(t == 0), stop=(t == 7))
                    adds = addt[:, dt * 1024:(dt + 1) * 1024]
                    nc.vector.tensor_scalar_mul(adds, acc[:], recU[:, dt:dt + 1])
                    nc.vector.tensor_add(
                        adds, adds,
                        w1[:, 8192 + dt * 1024: 8192 + (dt + 1) * 1024])

                # ---- mask[ct] = sum_kt wmask(kt,ct)^T @ add[kt]; out = mask + x ----
                # Pre-mask prefetch: first mask weights + x-residual tiles, and
                # next-sample upper-X (deps clear per-slab during BT drains).
                wms, xts = {}, {}
                for ct in range(4):
                    wms[ct] = wcp.tile([128, 1024], f32r, tag="wcol",
                                       name=f"wm{u}_{ct}")
                    nc.sync.dma_start(wms[ct][:],
                                      wmask_t[ct].rearrange("p k m -> p (k m)"))
                for ct in range(3):
                    xts[ct] = xrp.tile([128, 1024], f32, tag="xr", name=f"xr{u}_{ct}")
                    nc.sync.dma_start(xts[ct][:], x_t[s, ct * 128:(ct + 1) * 128, :])
                if s_nxt is not None:
                    for ct in range(8, 16):
                        emit_x_load(s_nxt, ct)
                for ct in range(16):
                    wm = wms.pop(ct)
                    if ct + 4 < 16:
                        wms[ct + 4] = wcp.tile([128, 1024], f32r, tag="wcol",
                                               name=f"wm{u}_{ct + 4}")
                        nc.sync.dma_start(wms[ct + 4][:],
                                          wmask_t[ct + 4].rearrange("p k m -> p (k m)"))
                    xt = xts.pop(ct)
                    if ct + 3 < 16:
                        xts[ct + 3] = xrp.tile([128, 1024], f32, tag="xr",
                                               name=f"xr{u}_{ct + 3}")
                        nc.sync.dma_start(xts[ct + 3][:],
                                          x_t[s, (ct + 3) * 128:(ct + 4) * 128, :])
                    acc = psa.tile([128, 1024], f32, tag="acc", name=f"mkacc{u}_{ct}")
                    for kt in range(8):
                        for ch in range(2):
                            nc.tensor.matmul(
                                acc[:, ch * 512:(ch + 1) * 512],
                                wm[:, kt * 128:(kt + 1) * 128],
                                addt[:, kt * 1024 + ch * 512:
                                     kt * 1024 + (ch + 1) * 512],
                                start=(kt == 0), stop=(kt == 7))
                    nc.vector.tensor_add(xt[:], acc[:], xt[:])
                    nc.scalar.dma_start(out_t[s, ct * 128:(ct + 1) * 128, :], xt[:])
                if s_nxt is not None:
                    nxt[u + 1] = {mt: emit_wp(u + 1, mt) for mt in range(2)}

    nc.compile()
    _PROG[reps] = nc
    return nc


def kernel(x, w_phi, w_eca_q, w_theta, w_eca_k, w_mask):
    from concourse.bass_utils import run_bass_kernel_spmd

    x = np.asarray(x, np.float32)
    w_phi = np.asarray(w_phi, np.float32)
    w_mask = np.asarray(w_mask, np.float32)
    w_eca_q = np.asarray(w_eca_q, np.float32)

    # host-side weight re-layouts (tiled for efficient per-partition DMA)
    # wphi[mt, p, kt, m] = w_phi[mt*128+m, kt*128+p]
    wphi_l = np.ascontiguousarray(
        w_phi.reshape(8, 128, 16, 128).transpose(0, 3, 2, 1))
    # wmask[ct, p, kt, m] = w_mask[ct*128+m, kt*128+p]
    wmask_l = np.ascontiguousarray(
        w_mask.reshape(16, 128, 8, 128).transpose(0, 3, 2, 1))
    bands = _make_bands(w_eca_q)

    xs = x.reshape(_NCORES, _SPC, _C, _N)
    nc = _build()
    in_maps = [{"x": np.ascontiguousarray(xs[i]), "wphi": wphi_l,
                "wmask": wmask_l, "bands": bands} for i in range(_NCORES)]
    res = run_bass_kernel_spmd(nc, in_maps, list(range(_NCORES)))
    out = np.stack([res.results[i]["out"] for i in range(_NCORES)])
    return out.reshape(_NCORES * _SPC, _C, _H, _H)
